# revision 6
# baseline (speedup 1.0000x reference)
"""Trainium2 Bass kernel for nn_CorrectSplineLinear (embedding_lookup regime).

Math: reference computes
    W[o,t,f] = sum_c interp[o,t,c] * E[c,f]        (interp = piecewise-linear in t)
    out[o,b,t] = sum_f x[b,f] * W[o,t,f]
which collapses algebraically to
    y[c,b]    = sum_f E[c,f] * x[b,f]              ([128,128] matmul)
    Z[o,s,b]  = sum_c cv[o,s,c] * y[c,b]           ([128,128] matmul per core)
    out[o,b,t]= Z[o,j(t),b] + tl(t)*(Z[o,j(t)+1,b] - Z[o,j(t),b])
so no [O,I,I] weight is ever materialized.  The kernel is memory-bound on
writing the [256,128,512] fp32 output (8 MiB per core across 8 cores).

Sharding: out_features O=256 split across 8 cores (32 rows each); x and E
replicated; each core gets its control_values slice pre-transposed.

The expansion (one tensor_scalar/activation per spline segment per output
row: out = tl*dZ + Z with two per-partition scalars) is spread across
VectorE, ScalarE, and GpSimdE so the output DMA stream, not compute, is
the pacing resource.
"""

import sys
from contextlib import ExitStack

import numpy as np

try:
    import concourse.bass as bass
except ImportError:  # fresh grading dir: concourse lives in the repo checkout
    sys.path.insert(0, "/opt/trn_rl_repo")
    import concourse.bass as bass

import concourse.bacc as bacc
import concourse.mybir as mybir
import concourse.tile as tile
from concourse.bass_utils import run_bass_kernel_spmd

N_CORES = 8
O, I, K, C, B = 256, 512, 3, 128, 128
OL = O // N_CORES  # 32 output rows per core
NS = K + 1  # 4 control values per output row
F32 = mybir.dt.float32

# ---- spline geometry (input-independent, mirrors reference arithmetic) ----
_t = np.linspace(0.0, 1.0, I).astype(np.float32)
_ts = (_t * np.float32(K)).astype(np.float32)
_j = np.clip(np.floor(_ts), 0.0, float(K - 1)).astype(np.int32)
_TL = (_ts - _j.astype(np.float32)).astype(np.float32)  # [I] local coord in segment
_b0 = int(np.searchsorted(_j, 1))  # first t index in segment 1
_b1 = int(np.searchsorted(_j, 2))  # first t index in segment 2
# Disjoint per-segment spans; each output row's three segment ops run on
# three different engines in parallel (VectorE / ScalarE / GpSimdE).
_SPANS = [(0, 0, _b0), (1, _b0, _b1), (2, _b1, I)]  # (segment j, t0, t1)
_SPAN_ENG = ["a", "v", "g"]  # engine per segment: ScalarE, VectorE, GpSimdE

# ---- packed-input column layout ([128, _TOT] fp32) ----
# 4 chunk-pairs [xT_k | eT_k] so matmul k can start as soon as chunk k lands,
# then cvT, then tl.
_CH0 = 0  # chunk k at [k*256, k*256+256): xT_k cols 0:128, eT_k cols 128:256
_CV0 = 4 * (B + C)  # cv slab transposed: [c, o*4+s]
_TL0 = _CV0 + OL * NS  # tl broadcast to 128 partitions
_TOT = _TL0 + I

GROUP = 4  # output rows per store DMA (4*128*512*4B = 1 MiB)
NGRP = OL // GROUP
EARLY_GROUPS = 2  # first groups store per-row (256KB) so the write stream starts ASAP
N_WARM = 8  # dummy matmuls to lift the PE HAM clock gate before the real chain

_cache: dict = {}


def _build_nc():
    nc = bacc.Bacc("TRN2", target_bir_lowering=False, debug=False, num_devices=N_CORES)
    pk_d = nc.dram_tensor("pk", [128, _TOT], F32, kind="ExternalInput")
    out_d = nc.dram_tensor("out", [OL, B, I], F32, kind="ExternalOutput")

    with tile.TileContext(nc) as tc, ExitStack() as ctx:
        constp = ctx.enter_context(tc.tile_pool(name="const", bufs=1))
        psump = ctx.enter_context(
            tc.tile_pool(name="psum", bufs=1, space=bass.MemorySpace.PSUM)
        )
        outp = ctx.enter_context(tc.tile_pool(name="outs", bufs=1))

        pk = constp.tile([128, _TOT], F32)
        # chunked input loads: matmul k only waits for its own 128KB
        for k in range(4):
            nc.sync.dma_start(
                pk[:, k * 256 : (k + 1) * 256], pk_d[:, k * 256 : (k + 1) * 256]
            )
        nc.sync.dma_start(pk[:, _CV0:_TL0], pk_d[:, _CV0:_TL0])
        nc.sync.dma_start(pk[:, _TL0:_TOT], pk_d[:, _TL0:_TOT])

        # PE warm-up: dummy matmuls while the input DMA is in flight keep the
        # HAM activity window busy so the real chain runs at 2.4 GHz.
        warm = constp.tile([128, I], F32)
        nc.gpsimd.memset(warm[:], 0.0)
        warm_ps = psump.tile([2, I], F32)
        for _ in range(N_WARM):
            nc.tensor.matmul(warm_ps[:], warm[:, :2], warm[:], start=True, stop=True)

        # y[c,b] = sum_f E[c,f] x[b,f]: accumulate over 4 chunks of f.
        y_ps = psump.tile([128, B], F32)
        for k in range(4):
            base = k * 256
            nc.tensor.matmul(
                y_ps[:],
                pk[:, base + B : base + B + C],  # lhsT [f_chunk, c]
                pk[:, base : base + B],  # rhs  [f_chunk, b]
                start=(k == 0),
                stop=(k == 3),
            )
        y_sb = constp.tile([128, B], F32)
        # ScalarE Identity == copy; same ACT table set as the expansion ops
        nc.scalar.activation(y_sb[:], y_ps[:], mybir.ActivationFunctionType.Identity)

        # ZT[b, o*4+s] = sum_c y[c,b] cvT[c, o*4+s]
        zt_ps = psump.tile([128, OL * NS], F32)
        nc.tensor.matmul(
            zt_ps[:], y_sb[:], pk[:, _CV0 : _CV0 + OL * NS], start=True, stop=True
        )
        zt = constp.tile([128, OL * NS], F32)
        nc.scalar.activation(zt[:], zt_ps[:], mybir.ActivationFunctionType.Identity)
        dzt = constp.tile([128, OL * NS], F32)
        nc.gpsimd.tensor_sub(
            dzt[:, 0 : OL * NS - 1], zt[:, 1 : OL * NS], zt[:, 0 : OL * NS - 1]
        )

        outs = outp.tile([128, OL * I], F32)
        tl_ap = pk[:, _TL0 : _TL0 + I]

        for g in range(NGRP):
            for oi in range(GROUP):
                o = g * GROUP + oi
                col = o * I
                zc = NS * o
                for (j, t0, t1), eng in zip(_SPANS, _SPAN_ENG):
                    if eng == "a":
                        nc.scalar.activation(
                            outs[:, col + t0 : col + t1],
                            tl_ap[:, t0:t1],
                            mybir.ActivationFunctionType.Identity,
                            bias=zt[:, zc + j : zc + j + 1],
                            scale=dzt[:, zc + j : zc + j + 1],
                        )
                    else:
                        veng = nc.vector if eng == "v" else nc.gpsimd
                        veng.tensor_scalar(
                            outs[:, col + t0 : col + t1],
                            tl_ap[:, t0:t1],
                            dzt[:, zc + j : zc + j + 1],
                            zt[:, zc + j : zc + j + 1],
                            mybir.AluOpType.mult,
                            mybir.AluOpType.add,
                        )
                if g < EARLY_GROUPS:
                    nc.sync.dma_start(
                        out_d[o : o + 1].rearrange("o b t -> b o t"),
                        outs[:, o * I : (o + 1) * I].rearrange("p (o t) -> p o t", o=1),
                    )
            if g >= EARLY_GROUPS:
                nc.sync.dma_start(
                    out_d[g * GROUP : (g + 1) * GROUP].rearrange("o b t -> b o t"),
                    outs[:, g * GROUP * I : (g + 1) * GROUP * I].rearrange(
                        "p (o t) -> p o t", o=GROUP
                    ),
                )

    nc.compile()
    return nc


def _get_nc():
    if "nc" not in _cache:
        _cache["nc"] = _build_nc()
    return _cache["nc"]


def _pack_inputs(x, control_values, expansion_matrix):
    x = np.ascontiguousarray(x, dtype=np.float32)
    cv = np.ascontiguousarray(control_values, dtype=np.float32)
    E = np.ascontiguousarray(expansion_matrix, dtype=np.float32)

    base = np.empty((128, _TOT), dtype=np.float32)
    for k in range(4):
        base[:, k * 256 : k * 256 + B] = x[:, k * 128 : (k + 1) * 128].T
        base[:, k * 256 + B : k * 256 + B + C] = E[:, k * 128 : (k + 1) * 128].T
    base[:, _TL0 : _TL0 + I] = _TL[None, :]

    in_maps = []
    for core in range(N_CORES):
        m = base.copy()
        slab = cv[core * OL : (core + 1) * OL].reshape(OL * NS, C)  # [(o,s), c]
        m[:, _CV0 : _CV0 + OL * NS] = slab.T
        in_maps.append({"pk": m})
    return in_maps


def _run(in_maps, trace=False):
    nc = _get_nc()
    return run_bass_kernel_spmd(
        nc, in_maps, core_ids=list(range(N_CORES)), trace=trace
    )


def kernel(x, control_points, control_values, expansion_matrix):
    in_maps = _pack_inputs(x, control_values, expansion_matrix)
    res = _run(in_maps, trace=False)
    return np.concatenate([r["out"] for r in res.results], axis=0)


def kernel_traced(x, control_points, control_values, expansion_matrix):
    """Same as kernel() but profiles on HW; returns (out, BassKernelResults)."""
    in_maps = _pack_inputs(x, control_values, expansion_matrix)
    res = _run(in_maps, trace=True)
    out = np.concatenate([r["out"] for r in res.results], axis=0)
    return out, res


# revision 8
# speedup vs baseline: 1.2003x; 1.2003x over previous
"""Trainium2 Bass kernel for nn_CorrectSplineLinear (embedding_lookup regime).

Math: reference computes
    W[o,t,f] = sum_c interp[o,t,c] * E[c,f]        (interp = piecewise-linear in t)
    out[o,b,t] = sum_f x[b,f] * W[o,t,f]
which collapses algebraically to
    y[c,b]    = sum_f E[c,f] * x[b,f]              ([128,128] matmul)
    Z[o,s,b]  = sum_c cv[o,s,c] * y[c,b]           ([128,128] matmul per core)
    out[o,b,t]= Z[o,j(t),b] + tl(t)*(Z[o,j(t)+1,b] - Z[o,j(t),b])
so no [O,I,I] weight is ever materialized.  The kernel is memory-bound on
writing the [256,128,512] fp32 output (8 MiB per core across 8 cores).

Sharding: out_features O=256 split across 8 cores (32 rows each); x and E
replicated; each core gets its control_values slice pre-transposed.

The expansion (one tensor_scalar/activation per spline segment per output
row: out = tl*dZ + Z with two per-partition scalars) is spread across
VectorE, ScalarE, and GpSimdE so the output DMA stream, not compute, is
the pacing resource.
"""

import sys
from contextlib import ExitStack

import numpy as np

try:
    import concourse.bass as bass
except ImportError:  # fresh grading dir: concourse lives in the repo checkout
    sys.path.insert(0, "/opt/trn_rl_repo")
    import concourse.bass as bass

import concourse.bacc as bacc
import concourse.mybir as mybir
import concourse.tile as tile
from concourse.bass_utils import run_bass_kernel_spmd

N_CORES = 8
O, I, K, C, B = 256, 512, 3, 128, 128
OL = O // N_CORES  # 32 output rows per core
NS = K + 1  # 4 control values per output row
F32 = mybir.dt.float32

# ---- spline geometry (input-independent, mirrors reference arithmetic) ----
_t = np.linspace(0.0, 1.0, I).astype(np.float32)
_ts = (_t * np.float32(K)).astype(np.float32)
_j = np.clip(np.floor(_ts), 0.0, float(K - 1)).astype(np.int32)
_TL = (_ts - _j.astype(np.float32)).astype(np.float32)  # [I] local coord in segment
_b0 = int(np.searchsorted(_j, 1))  # first t index in segment 1
_b1 = int(np.searchsorted(_j, 2))  # first t index in segment 2
# Disjoint per-segment spans; each output row's three segment ops run on
# three different engines in parallel (VectorE / ScalarE / GpSimdE).
_SPANS = [(0, 0, _b0), (1, _b0, _b1), (2, _b1, I)]  # (segment j, t0, t1)
_SPAN_ENG = ["a", "v", "g"]  # engine per segment: ScalarE, VectorE, GpSimdE

# ---- packed-input column layout ([128, _TOT] fp32) ----
# 4 chunk-pairs [xT_k | eT_k] so matmul k can start as soon as chunk k lands,
# then cvT, then tl.
_CH0 = 0  # chunk k at [k*256, k*256+256): xT_k cols 0:128, eT_k cols 128:256
_CV0 = 4 * (B + C)  # cv slab transposed: [c, o*4+s]
_TL0 = _CV0 + OL * NS  # tl broadcast to 128 partitions
_TOT = _TL0 + I

GROUP = 4  # output rows per store DMA (4*128*512*4B = 1 MiB)
NGRP = OL // GROUP
EARLY_GROUPS = 2  # first groups store per-row (256KB) so the write stream starts ASAP
ZT_SPLIT = NS * GROUP  # zt columns needed by the first store group

_cache: dict = {}


def _build_nc():
    nc = bacc.Bacc("TRN2", target_bir_lowering=False, debug=False, num_devices=N_CORES)
    pk_d = nc.dram_tensor("pk", [128, _TOT], F32, kind="ExternalInput")
    out_d = nc.dram_tensor("out", [OL, B, I], F32, kind="ExternalOutput")

    with tile.TileContext(nc) as tc, ExitStack() as ctx:
        constp = ctx.enter_context(tc.tile_pool(name="const", bufs=1))
        psump = ctx.enter_context(
            tc.tile_pool(name="psum", bufs=1, space=bass.MemorySpace.PSUM)
        )
        outp = ctx.enter_context(tc.tile_pool(name="outs", bufs=1))

        pk = constp.tile([128, _TOT], F32)
        # input loads on ScalarE's HWDGE ring so SyncE is free for the output
        # stream; 2-chunk granularity so matmuls start as data arrives
        nc.scalar.dma_start(pk[:, 0:512], pk_d[:, 0:512])
        nc.scalar.dma_start(pk[:, 512:1024], pk_d[:, 512:1024])
        nc.scalar.dma_start(pk[:, _CV0:_TOT], pk_d[:, _CV0:_TOT])

        # y[c,b] = sum_f E[c,f] x[b,f]: accumulate over 4 chunks of f.
        y_ps = psump.tile([128, B], F32)
        for k in range(4):
            base = k * 256
            nc.tensor.matmul(
                y_ps[:],
                pk[:, base + B : base + B + C],  # lhsT [f_chunk, c]
                pk[:, base : base + B],  # rhs  [f_chunk, b]
                start=(k == 0),
                stop=(k == 3),
            )
        y_sb = constp.tile([128, B], F32)
        nc.vector.tensor_copy(y_sb[:], y_ps[:])

        # ZT[b, o*4+s] = sum_c y[c,b] cvT[c, o*4+s]; the first ZT_SPLIT
        # columns go in a separate matmul so group 0 unblocks early.
        zt_ps = psump.tile([128, OL * NS], F32)
        zt = constp.tile([128, OL * NS], F32)
        dzt = constp.tile([128, OL * NS], F32)
        nc.tensor.matmul(
            zt_ps[:, :ZT_SPLIT],
            y_sb[:],
            pk[:, _CV0 : _CV0 + ZT_SPLIT],
            start=True,
            stop=True,
        )
        nc.vector.tensor_copy(zt[:, :ZT_SPLIT], zt_ps[:, :ZT_SPLIT])
        nc.gpsimd.tensor_sub(
            dzt[:, : ZT_SPLIT - 1], zt[:, 1:ZT_SPLIT], zt[:, : ZT_SPLIT - 1]
        )
        nc.tensor.matmul(
            zt_ps[:, ZT_SPLIT:],
            y_sb[:],
            pk[:, _CV0 + ZT_SPLIT : _CV0 + OL * NS],
            start=True,
            stop=True,
        )
        nc.vector.tensor_copy(zt[:, ZT_SPLIT:], zt_ps[:, ZT_SPLIT:])
        nc.gpsimd.tensor_sub(
            dzt[:, ZT_SPLIT - 1 : OL * NS - 1],
            zt[:, ZT_SPLIT : OL * NS],
            zt[:, ZT_SPLIT - 1 : OL * NS - 1],
        )

        outs = outp.tile([128, OL * I], F32)
        tl_ap = pk[:, _TL0 : _TL0 + I]

        for g in range(NGRP):
            for oi in range(GROUP):
                o = g * GROUP + oi
                col = o * I
                zc = NS * o
                for (j, t0, t1), eng in zip(_SPANS, _SPAN_ENG):
                    if eng == "a":
                        nc.scalar.activation(
                            outs[:, col + t0 : col + t1],
                            tl_ap[:, t0:t1],
                            mybir.ActivationFunctionType.Identity,
                            bias=zt[:, zc + j : zc + j + 1],
                            scale=dzt[:, zc + j : zc + j + 1],
                        )
                    else:
                        veng = nc.vector if eng == "v" else nc.gpsimd
                        veng.tensor_scalar(
                            outs[:, col + t0 : col + t1],
                            tl_ap[:, t0:t1],
                            dzt[:, zc + j : zc + j + 1],
                            zt[:, zc + j : zc + j + 1],
                            mybir.AluOpType.mult,
                            mybir.AluOpType.add,
                        )
                if g < EARLY_GROUPS:
                    nc.sync.dma_start(
                        out_d[o : o + 1].rearrange("o b t -> b o t"),
                        outs[:, o * I : (o + 1) * I].rearrange("p (o t) -> p o t", o=1),
                    )
            if g >= EARLY_GROUPS:
                nc.sync.dma_start(
                    out_d[g * GROUP : (g + 1) * GROUP].rearrange("o b t -> b o t"),
                    outs[:, g * GROUP * I : (g + 1) * GROUP * I].rearrange(
                        "p (o t) -> p o t", o=GROUP
                    ),
                )

    nc.compile()
    return nc


def _get_nc():
    if "nc" not in _cache:
        _cache["nc"] = _build_nc()
    return _cache["nc"]


def _pack_inputs(x, control_values, expansion_matrix):
    x = np.ascontiguousarray(x, dtype=np.float32)
    cv = np.ascontiguousarray(control_values, dtype=np.float32)
    E = np.ascontiguousarray(expansion_matrix, dtype=np.float32)

    base = np.empty((128, _TOT), dtype=np.float32)
    for k in range(4):
        base[:, k * 256 : k * 256 + B] = x[:, k * 128 : (k + 1) * 128].T
        base[:, k * 256 + B : k * 256 + B + C] = E[:, k * 128 : (k + 1) * 128].T
    base[:, _TL0 : _TL0 + I] = _TL[None, :]

    in_maps = []
    for core in range(N_CORES):
        m = base.copy()
        slab = cv[core * OL : (core + 1) * OL].reshape(OL * NS, C)  # [(o,s), c]
        m[:, _CV0 : _CV0 + OL * NS] = slab.T
        in_maps.append({"pk": m})
    return in_maps


def _run(in_maps, trace=False):
    nc = _get_nc()
    return run_bass_kernel_spmd(
        nc, in_maps, core_ids=list(range(N_CORES)), trace=trace
    )


def kernel(x, control_points, control_values, expansion_matrix):
    in_maps = _pack_inputs(x, control_values, expansion_matrix)
    res = _run(in_maps, trace=False)
    return np.concatenate([r["out"] for r in res.results], axis=0)


def kernel_traced(x, control_points, control_values, expansion_matrix):
    """Same as kernel() but profiles on HW; returns (out, BassKernelResults)."""
    in_maps = _pack_inputs(x, control_values, expansion_matrix)
    res = _run(in_maps, trace=True)
    out = np.concatenate([r["out"] for r in res.results], axis=0)
    return out, res


# revision 12
# speedup vs baseline: 1.2074x; 1.0059x over previous
"""Trainium2 Bass kernel for nn_CorrectSplineLinear (embedding_lookup regime).

Math: reference computes
    W[o,t,f] = sum_c interp[o,t,c] * E[c,f]        (interp = piecewise-linear in t)
    out[o,b,t] = sum_f x[b,f] * W[o,t,f]
which collapses algebraically to
    y[c,b]    = sum_f E[c,f] * x[b,f]              ([128,128] matmul)
    Z[o,s,b]  = sum_c cv[o,s,c] * y[c,b]           ([128,128] matmul per core)
    out[o,b,t]= Z[o,j(t),b] + tl(t)*(Z[o,j(t)+1,b] - Z[o,j(t),b])
so no [O,I,I] weight is ever materialized.  The kernel is memory-bound on
writing the [256,128,512] fp32 output (8 MiB per core across 8 cores).

Sharding: out_features O=256 split across 8 cores (32 rows each); x and E
replicated; each core gets its control_values slice pre-transposed.

The expansion (one tensor_scalar/activation per spline segment per output
row: out = tl*dZ + Z with two per-partition scalars) is spread across
VectorE, ScalarE, and GpSimdE so the output DMA stream, not compute, is
the pacing resource.
"""

import sys
from contextlib import ExitStack

import numpy as np

try:
    import concourse.bass as bass
except ImportError:  # fresh grading dir: concourse lives in the repo checkout
    sys.path.insert(0, "/opt/trn_rl_repo")
    import concourse.bass as bass

import concourse.bacc as bacc
import concourse.mybir as mybir
import concourse.tile as tile
from concourse.bass_utils import run_bass_kernel_spmd

N_CORES = 8
O, I, K, C, B = 256, 512, 3, 128, 128
OL = O // N_CORES  # 32 output rows per core
NS = K + 1  # 4 control values per output row
F32 = mybir.dt.float32

# ---- spline geometry (input-independent, mirrors reference arithmetic) ----
_t = np.linspace(0.0, 1.0, I).astype(np.float32)
_ts = (_t * np.float32(K)).astype(np.float32)
_j = np.clip(np.floor(_ts), 0.0, float(K - 1)).astype(np.int32)
_TL = (_ts - _j.astype(np.float32)).astype(np.float32)  # [I] local coord in segment
_b0 = int(np.searchsorted(_j, 1))  # first t index in segment 1
_b1 = int(np.searchsorted(_j, 2))  # first t index in segment 2
# Disjoint per-segment spans; each output row's three segment ops run on
# three different engines in parallel (VectorE / ScalarE / GpSimdE).
_SPANS = [(0, 0, _b0), (1, _b0, _b1), (2, _b1, I)]  # (segment j, t0, t1)
_SPAN_ENG = ["a", "v", "g"]  # engine per segment: ScalarE, VectorE, GpSimdE

# ---- packed-input column layout ([128, _TOT] fp32) ----
# 4 chunk-pairs [xT_k | eT_k] so matmul k can start as soon as chunk k lands,
# then cvT, then tl.
_CH0 = 0  # chunk k at [k*256, k*256+256): xT_k cols 0:128, eT_k cols 128:256
_CV0 = 4 * (B + C)  # cv slab transposed: [c, o*4+s]
_TL0 = _CV0 + OL * NS  # tl broadcast to 128 partitions
_TOT = _TL0 + I

GROUP = 4  # output rows per store DMA (4*128*512*4B = 1 MiB)
NGRP = OL // GROUP
EARLY_GROUPS = 2  # first groups store per-row (256KB) so the write stream starts ASAP
ZT_SPLIT = NS * GROUP  # zt columns needed by the first store group

_cache: dict = {}


def _build_nc():
    nc = bacc.Bacc("TRN2", target_bir_lowering=False, debug=False, num_devices=N_CORES)
    pk_d = nc.dram_tensor("pk", [128, _TOT], F32, kind="ExternalInput")
    out_d = nc.dram_tensor("out", [OL, B, I], F32, kind="ExternalOutput")

    with tile.TileContext(nc) as tc, ExitStack() as ctx:
        constp = ctx.enter_context(tc.tile_pool(name="const", bufs=1))
        psump = ctx.enter_context(
            tc.tile_pool(name="psum", bufs=1, space=bass.MemorySpace.PSUM)
        )
        outp = ctx.enter_context(tc.tile_pool(name="outs", bufs=1))

        pk = constp.tile([128, _TOT], F32)
        # input loads spread across three DMA paths so their issue overhead
        # runs in parallel and matmuls start as early as possible
        nc.sync.dma_start(pk[:, 0:512], pk_d[:, 0:512])
        nc.gpsimd.dma_start(pk[:, 512:1024], pk_d[:, 512:1024])
        nc.scalar.dma_start(pk[:, _CV0:_TOT], pk_d[:, _CV0:_TOT])

        # PE warm-up: tiny bf16 dummy matmuls while the input DMA is in
        # flight lift the HAM clock gate so the real chain runs at 2.4 GHz.
        BF16 = mybir.dt.bfloat16
        warm = constp.tile([128, 64], BF16)
        nc.vector.memset(warm[:], 0.0)
        warm_ps = psump.tile([1, 64], F32)
        for _ in range(36):
            nc.tensor.matmul(warm_ps[:], warm[:, :1], warm[:], start=True, stop=True)

        # y[c,b] = sum_f E[c,f] x[b,f]: accumulate over 4 chunks of f.
        y_ps = psump.tile([128, B], F32)
        for k in range(4):
            base = k * 256
            nc.tensor.matmul(
                y_ps[:],
                pk[:, base + B : base + B + C],  # lhsT [f_chunk, c]
                pk[:, base : base + B],  # rhs  [f_chunk, b]
                start=(k == 0),
                stop=(k == 3),
            )
        y_sb = constp.tile([128, B], F32)
        nc.vector.tensor_copy(y_sb[:], y_ps[:])

        # ZT[b, o*4+s] = sum_c y[c,b] cvT[c, o*4+s]; the first ZT_SPLIT
        # columns go in a separate matmul so group 0 unblocks early.
        # Chain ops stay on VectorE to avoid cross-engine hops; the rest of
        # dzt is computed on GpSimd ahead of its group-1 expansion ops.
        zt_ps = psump.tile([128, ZT_SPLIT], F32)
        zt_ps2 = psump.tile([128, OL * NS - ZT_SPLIT], F32)
        zt = constp.tile([128, OL * NS], F32)
        dzt = constp.tile([128, OL * NS], F32)
        nc.tensor.matmul(
            zt_ps[:],
            y_sb[:],
            pk[:, _CV0 : _CV0 + ZT_SPLIT],
            start=True,
            stop=True,
        )
        nc.vector.tensor_copy(zt[:, :ZT_SPLIT], zt_ps[:])
        nc.vector.tensor_sub(
            dzt[:, : ZT_SPLIT - 1], zt[:, 1:ZT_SPLIT], zt[:, : ZT_SPLIT - 1]
        )

        def _zt_rest():
            nc.tensor.matmul(
                zt_ps2[:],
                y_sb[:],
                pk[:, _CV0 + ZT_SPLIT : _CV0 + OL * NS],
                start=True,
                stop=True,
            )
            nc.scalar.activation(
                zt[:, ZT_SPLIT:],
                zt_ps2[:],
                mybir.ActivationFunctionType.Identity,
            )
            nc.gpsimd.tensor_sub(
                dzt[:, ZT_SPLIT - 1 : OL * NS - 1],
                zt[:, ZT_SPLIT : OL * NS],
                zt[:, ZT_SPLIT - 1 : OL * NS - 1],
            )

        outs = outp.tile([128, OL * I], F32)
        tl_ap = pk[:, _TL0 : _TL0 + I]

        for g in range(NGRP):
            if g == 1:
                _zt_rest()
            for oi in range(GROUP):
                o = g * GROUP + oi
                col = o * I
                zc = NS * o
                for (j, t0, t1), eng in zip(_SPANS, _SPAN_ENG):
                    if eng == "a":
                        nc.scalar.activation(
                            outs[:, col + t0 : col + t1],
                            tl_ap[:, t0:t1],
                            mybir.ActivationFunctionType.Identity,
                            bias=zt[:, zc + j : zc + j + 1],
                            scale=dzt[:, zc + j : zc + j + 1],
                        )
                    else:
                        veng = nc.vector if eng == "v" else nc.gpsimd
                        veng.tensor_scalar(
                            outs[:, col + t0 : col + t1],
                            tl_ap[:, t0:t1],
                            dzt[:, zc + j : zc + j + 1],
                            zt[:, zc + j : zc + j + 1],
                            mybir.AluOpType.mult,
                            mybir.AluOpType.add,
                        )
                if g < EARLY_GROUPS:
                    nc.sync.dma_start(
                        out_d[o : o + 1].rearrange("o b t -> b o t"),
                        outs[:, o * I : (o + 1) * I].rearrange("p (o t) -> p o t", o=1),
                    )
            if g >= EARLY_GROUPS:
                nc.sync.dma_start(
                    out_d[g * GROUP : (g + 1) * GROUP].rearrange("o b t -> b o t"),
                    outs[:, g * GROUP * I : (g + 1) * GROUP * I].rearrange(
                        "p (o t) -> p o t", o=GROUP
                    ),
                )

    nc.compile()
    return nc


def _get_nc():
    if "nc" not in _cache:
        _cache["nc"] = _build_nc()
    return _cache["nc"]


def _pack_inputs(x, control_values, expansion_matrix):
    x = np.ascontiguousarray(x, dtype=np.float32)
    cv = np.ascontiguousarray(control_values, dtype=np.float32)
    E = np.ascontiguousarray(expansion_matrix, dtype=np.float32)

    base = np.empty((128, _TOT), dtype=np.float32)
    for k in range(4):
        base[:, k * 256 : k * 256 + B] = x[:, k * 128 : (k + 1) * 128].T
        base[:, k * 256 + B : k * 256 + B + C] = E[:, k * 128 : (k + 1) * 128].T
    base[:, _TL0 : _TL0 + I] = _TL[None, :]

    in_maps = []
    for core in range(N_CORES):
        m = base.copy()
        slab = cv[core * OL : (core + 1) * OL].reshape(OL * NS, C)  # [(o,s), c]
        m[:, _CV0 : _CV0 + OL * NS] = slab.T
        in_maps.append({"pk": m})
    return in_maps


def _run(in_maps, trace=False):
    nc = _get_nc()
    return run_bass_kernel_spmd(
        nc, in_maps, core_ids=list(range(N_CORES)), trace=trace
    )


def kernel(x, control_points, control_values, expansion_matrix):
    in_maps = _pack_inputs(x, control_values, expansion_matrix)
    res = _run(in_maps, trace=False)
    return np.concatenate([r["out"] for r in res.results], axis=0)


def kernel_traced(x, control_points, control_values, expansion_matrix):
    """Same as kernel() but profiles on HW; returns (out, BassKernelResults)."""
    in_maps = _pack_inputs(x, control_values, expansion_matrix)
    res = _run(in_maps, trace=True)
    out = np.concatenate([r["out"] for r in res.results], axis=0)
    return out, res
